# revision 31
# baseline (speedup 1.0000x reference)
"""APNB (asymmetric pyramid non-local block) on 8 TRN2 NeuronCores.

Data-parallel: one batch sample per core. Per core (x: [512, 9216] of one
sample):

  Algorithmic restructure: psp_pool(conv1x1(x, W, b)) == W @ psp_pool(x) + b
  (both linear), so the k/v convolutions over the full 96x96 image collapse
  to tiny matmuls on the 110 pooled vectors.

  Pass 1 (streams x + xT from HBM, bf16):
    - q = Wq @ x + bq                  (PE, per chunk; kept resident bf16)
    - pooledT = Mpool.T @ xT           (PE, accumulating in one PSUM bank)
  Finalize:
    - pooled = pooledT.T               (PE transpose)
    - k_pool = Wk @ pooled + bk        (PE + ACT bias)
    - v_poolT = pooled.T @ Wv.T + bv   (PE, rank-1 bias matmul)
  Pass 2 (per 512-column chunk; streams out to HBM):
    - attnT  = k_pool.T @ q_chunk      [110, 512]  (PE)
    - exp    = exp(attnT)              (ACT, psum->sbuf bf16)
    - denom  = ones @ exp              (PE, replicated row sums)
    - attn   = exp * 1/denom           (DVE)
    - out    = v_poolT.T @ attn + I @ x_chunk   (PE, residual via identity
               matmul accumulated into the same PSUM bank)
    - copy psum -> sbuf (DVE/ACT) -> DMA out (fp32)

Softmax needs no max-subtraction: logits are in [-8, 8] for this problem
family (checked against the reference; exp stays finite in fp32).
"""

import numpy as np
import ml_dtypes

import concourse.bass as bass
import concourse.bacc as bacc
import concourse.tile as tile
import concourse.mybir as mybir
from concourse.bass_utils import run_bass_kernel_spmd

BF16 = ml_dtypes.bfloat16
AF = mybir.ActivationFunctionType

B = 8
C = 512          # input/value channels
O = 256          # q/k channels
H = 96
W = 96
N = H * W        # 9216
S = 110          # pooled length 1+9+36+64
PSP = (1, 3, 6, 8)
NCORES = 8
CHUNK = 1024     # columns per input DMA chunk
NBIG = N // CHUNK
SUB = 512        # columns per compute sub-chunk
KT = C // 128    # 4 contraction tiles over channels
MT_O = O // 128  # 2 output tiles for q/k
NT = N // 128    # 72 position tiles


def _build_pool_matrix() -> np.ndarray:
    m = np.zeros((N, S), dtype=np.float32)
    col = 0
    for s in PSP:
        hb, wb = H // s, W // s
        scale = 1.0 / (hb * wb)
        for i in range(s):
            for j in range(s):
                blk = np.zeros((H, W), np.float32)
                blk[i * hb:(i + 1) * hb, j * wb:(j + 1) * wb] = scale
                m[:, col] = blk.reshape(-1)
                col += 1
    assert col == S
    return m


def build_nc() -> bacc.Bacc:
    nc = bacc.Bacc("TRN2", target_bir_lowering=False, debug=False,
                   num_devices=NCORES)
    bf = mybir.dt.bfloat16
    f32 = mybir.dt.float32

    def din(name, shape, dt):
        return nc.dram_tensor(name, shape, dt, kind="ExternalInput").ap()

    x_d = din("x_bf", [C, N], bf)
    xt_d = din("xT_bf", [N, C], bf)
    mp_d = din("mpool", [N, S], bf)
    wq_d = din("wq_o", [O, C], bf)
    wk_d = din("wkT", [C, O], bf)
    wv_d = din("wvT", [C, C], bf)
    bq_d = din("bq_col", [O, 1], bf)
    bk_d = din("bk_bf", [1, O], bf)
    bv_d = din("bv_bf", [1, C], bf)
    ones_d = din("ones_bf", [128, 512], bf)
    id_d = din("ident_bf", [128, 128], bf)
    out_d = nc.dram_tensor("out", [C, N], f32, kind="ExternalOutput").ap()

    xv = x_d.rearrange("(g p) n -> p g n", p=128)      # [128, 4, N]
    xtv = xt_d.rearrange("(t p) c -> p t c", p=128)    # [128, 72, C]
    mpv = mp_d.rearrange("(t p) s -> p t s", p=128)    # [128, 72, S]
    wqv = wq_d.rearrange("(t p) c -> p t c", p=128)    # [128, 2, C]
    wkv = wk_d.rearrange("(k p) m -> p k m", p=128)
    wvv = wv_d.rearrange("(k p) m -> p k m", p=128)    # [128, 4, C]
    bqv = bq_d.rearrange("(t p) o -> p t o", p=128)    # [128, 2, 1]
    outv = out_d.rearrange("(g p) n -> p g n", p=128)  # [128, 4, N]

    from contextlib import ExitStack
    with tile.TileContext(nc) as tc, ExitStack() as ctx:
        consts = ctx.enter_context(tc.tile_pool(name="consts", bufs=1))
        resid = ctx.enter_context(tc.tile_pool(name="resid", bufs=1))

        # consts + xT go through gpsimd SWDGE so they never contend with the
        # x/out streams on the SP HWDGE queue, nor with ACT compute
        wq_sb = consts.tile([128, MT_O, C], bf)
        nc.gpsimd.dma_start(out=wq_sb, in_=wqv)
        bq_sb = consts.tile([128, MT_O, 1], bf)
        nc.gpsimd.dma_start(out=bq_sb, in_=bqv)
        ones_sb = consts.tile([128, 512], bf)
        nc.gpsimd.dma_start(out=ones_sb, in_=ones_d)
        mp_sb = consts.tile([128, NT, S], bf)
        nc.gpsimd.dma_start(out=mp_sb, in_=mpv)
        wk_sb = consts.tile([128, KT, O], bf)
        nc.gpsimd.dma_start(out=wk_sb, in_=wkv)
        wv_sb = consts.tile([128, KT, C], bf)
        nc.gpsimd.dma_start(out=wv_sb, in_=wvv)
        bk_sb = consts.tile([1, O], bf)
        nc.gpsimd.dma_start(out=bk_sb, in_=bk_d)
        bv_sb = consts.tile([1, C], bf)
        nc.gpsimd.dma_start(out=bv_sb, in_=bv_d)
        id_sb = consts.tile([128, 128], bf)
        nc.gpsimd.dma_start(out=id_sb, in_=id_d)

        x_sb = resid.tile([128, KT, N], bf)       # resident input, bf16

        k_pool_sb = consts.tile([128, MT_O, S], bf)
        vT_sb = consts.tile([110, C], bf)
        pooledT_sb = consts.tile([110, C], bf)
        pooled_sb = consts.tile([128, KT, S], bf)

        # ------- pass 1: stream x + xT, pool on PE (two alternating chains)
        with tc.tile_pool(name="p1ps", bufs=2, space="PSUM") as p1ps, \
             tc.tile_pool(name="poolps", bufs=1, space="PSUM") as poolps, \
             tc.tile_pool(name="xtp", bufs=3) as xtp:
            pooledT_psa = poolps.tile([110, C], mybir.dt.float32, tag="pa")
            pooledT_psb = poolps.tile([110, C], mybir.dt.float32, tag="pb")
            for ci in range(NBIG):
                nsl = slice(ci * CHUNK, (ci + 1) * CHUNK)
                nc.sync.dma_start(out=x_sb[:, :, nsl], in_=xv[:, :, nsl])
                xt_t = xtp.tile([128, CHUNK // 128, C], bf, tag="xt")
                nc.gpsimd.dma_start(
                    out=xt_t, in_=xtv[:, ci * (CHUNK // 128):(ci + 1) * (CHUNK // 128), :])
                for kt in range(CHUNK // 128):
                    kg = ci * (CHUNK // 128) + kt
                    acc = pooledT_psa if kg % 2 == 0 else pooledT_psb
                    nc.tensor.matmul(
                        acc, mp_sb[:, kg, :], xt_t[:, kt, :],
                        start=(kg < 2), stop=(kg >= NT - 2),
                        skip_group_check=True)

            # ---------------- finalize pools ----------------
            ptmp_sb = consts.tile([110, C], mybir.dt.float32)
            nc.scalar.copy(ptmp_sb, pooledT_psa)
            with nc.allow_low_precision("pooled stats stored bf16"):
                nc.vector.tensor_add(pooledT_sb, pooledT_psb, ptmp_sb)
            for c in range(KT):
                tr_ps = p1ps.tile([128, S], mybir.dt.bfloat16, tag="trps")
                nc.tensor.transpose(
                    tr_ps, pooledT_sb[:, c * 128:(c + 1) * 128],
                    id_sb[:110, :110])
                nc.scalar.copy(pooled_sb[:, c, :], tr_ps)
            kp_ps = p1ps.tile([128, MT_O, 128], mybir.dt.float32, tag="trps")
            for m in range(MT_O):
                for k in range(KT):
                    nc.tensor.matmul(
                        kp_ps[:, m, 0:S], wk_sb[:, k, m * 128:(m + 1) * 128],
                        pooled_sb[:, k, :], start=(k == 0), stop=False,
                        skip_group_check=True)
                nc.tensor.matmul(
                    kp_ps[:, m, 0:S], bk_sb[0:1, m * 128:(m + 1) * 128],
                    ones_sb[0:1, 0:S], start=False, stop=True,
                    skip_group_check=True)
            nc.scalar.copy(k_pool_sb, kp_ps[:, :, 0:S])
            # fold the query projection into the keys:
            #   attnT = (Wq.T k_pool).T @ x + (k_pool.T bq) * ones
            t_ps = p1ps.tile([110, 128], mybir.dt.float32, tag="tps")
            for kt in range(MT_O):
                nc.tensor.matmul(t_ps[:, 0:1], k_pool_sb[:, kt, :],
                                 bq_sb[:, kt, :],
                                 start=(kt == 0), stop=(kt == MT_O - 1),
                                 skip_group_check=True)
            t_sb = consts.tile([110, 1], mybir.dt.float32)
            nc.scalar.copy(t_sb, t_ps[:, 0:1])
            kq_sb = consts.tile([128, KT, S], bf)
            for cb in range(KT):
                kq_ps = p1ps.tile([128, 128], mybir.dt.float32, tag="tps")
                for kt in range(MT_O):
                    nc.tensor.matmul(
                        kq_ps[:, 0:S],
                        wq_sb[:, kt, cb * 128:(cb + 1) * 128],
                        k_pool_sb[:, kt, :],
                        start=(kt == 0), stop=(kt == MT_O - 1),
                        skip_group_check=True)
                nc.scalar.copy(kq_sb[:, cb, :], kq_ps[:, 0:S])
            vp_ps = p1ps.tile([110, C], mybir.dt.float32, tag="qps")
            for k in range(KT):
                nc.tensor.matmul(vp_ps, pooled_sb[:, k, :], wv_sb[:, k, :],
                                 start=(k == 0), stop=False,
                                 skip_group_check=True)
            nc.tensor.matmul(vp_ps, ones_sb[0:1, :110], bv_sb,
                             start=False, stop=True, skip_group_check=True)
            nc.scalar.copy(vT_sb, vp_ps)

        # ------- pass 2: folded attention + output, per 512-col chunk ----
        with tc.tile_pool(name="p2ps", bufs=2, space="PSUM") as p2ps, \
             tc.tile_pool(name="pops", bufs=3, space="PSUM") as pops, \
             tc.tile_pool(name="p2sb", bufs=3) as p2sb, \
             tc.tile_pool(name="outp", bufs=4) as outp:
            for ci in range(N // SUB):
                ns2 = slice(ci * SUB, (ci + 1) * SUB)
                at_ps = p2ps.tile([110, SUB], mybir.dt.float32, tag="atdb")
                for k in range(KT):
                    nc.tensor.matmul(at_ps, kq_sb[:, k, :],
                                     x_sb[:, k, ns2],
                                     start=(k == 0), stop=(k == KT - 1))
                exp_sb = p2sb.tile([110, SUB], mybir.dt.bfloat16, tag="exp")
                nc.scalar.activation(exp_sb, at_ps, AF.Exp,
                                     bias=t_sb, scale=1.0)
                db_ps = p2ps.tile([110, SUB], mybir.dt.float32, tag="atdb")
                nc.tensor.matmul(db_ps, ones_sb[:110, :110], exp_sb,
                                 start=True, stop=True)
                recip_sb = p2sb.tile([110, SUB], mybir.dt.float32, tag="recip")
                nc.vector.reciprocal_approx_fast(recip_sb, db_ps)
                attn_sb = p2sb.tile([110, SUB], mybir.dt.bfloat16, tag="attn")
                with nc.allow_low_precision("softmax weights tolerate bf16"):
                    nc.gpsimd.tensor_mul(attn_sb, exp_sb, recip_sb)
                out_t = outp.tile([128, KT, SUB], mybir.dt.float32, tag="out")
                # c 0,1: residual fused into the psum->sbuf move on DVE
                o_psa = pops.tile([128, 2, SUB], mybir.dt.float32, tag="ops")
                for c in range(2):
                    nc.tensor.matmul(o_psa[:, c, :],
                                     vT_sb[:, c * 128:(c + 1) * 128],
                                     attn_sb, start=True, stop=True,
                                     skip_group_check=True)
                with nc.allow_low_precision("fp32 psum + bf16 residual"):
                    nc.vector.tensor_add(out_t[:, 0:2, :], o_psa,
                                         x_sb[:, 0:2, ns2])
                # c 2,3: residual via identity matmul, copy on ACT
                o_psb = pops.tile([128, 2, SUB], mybir.dt.float32, tag="ops")
                for c in range(2, KT):
                    nc.tensor.matmul(o_psb[:, c - 2, :],
                                     vT_sb[:, c * 128:(c + 1) * 128],
                                     attn_sb, start=True, stop=False,
                                     skip_group_check=True)
                    nc.tensor.matmul(o_psb[:, c - 2, :], id_sb,
                                     x_sb[:, c, ns2],
                                     start=False, stop=True,
                                     skip_group_check=True)
                nc.scalar.copy(out_t[:, 2:4, :], o_psb)
                nc.gpsimd.dma_start(out=outv[:, :, ns2], in_=out_t)

    nc.compile()
    return nc


_NC_CACHE = None


def _get_nc() -> bacc.Bacc:
    global _NC_CACHE
    if _NC_CACHE is None:
        _NC_CACHE = build_nc()
    return _NC_CACHE


def _prep_in_maps(x, Wq, bq, Wk, bk, Wv, bv):
    shared = {
        "mpool": _build_pool_matrix().astype(BF16),
        "wq_o": np.ascontiguousarray(Wq).astype(BF16),
        "wkT": np.ascontiguousarray(Wk.T).astype(BF16),
        "wvT": np.ascontiguousarray(Wv.T).astype(BF16),
        "bq_col": np.ascontiguousarray(bq.reshape(O, 1)).astype(BF16),
        "bk_bf": np.ascontiguousarray(bk.reshape(1, O)).astype(BF16),
        "bv_bf": np.ascontiguousarray(bv.reshape(1, C)).astype(BF16),
        "ones_bf": np.ones((128, 512), dtype=BF16),
        "ident_bf": np.eye(128, dtype=np.float32).astype(BF16),
    }
    in_maps = []
    for i in range(NCORES):
        xi = np.ascontiguousarray(x[i].reshape(C, N))
        m = dict(shared)
        m["x_bf"] = xi.astype(BF16)
        m["xT_bf"] = np.ascontiguousarray(xi.T).astype(BF16)
        in_maps.append(m)
    return in_maps


def _install_ntff_hook():
    """The agent image ships no antenv.axon_hooks module, so trace=True
    under axon crashes on import. Recreate the tiny hook-holder module and
    register trn_boot's ctypes NTFF hook so neuron-profile timing works."""
    import sys
    import types
    if "antenv.axon_hooks" in sys.modules:
        return
    mod = types.ModuleType("antenv.axon_hooks")
    holder = {"h": None}
    mod.set_axon_ntff_profile_hook = lambda h: holder.__setitem__("h", h)
    mod.get_axon_ntff_profile_hook = lambda: holder["h"]
    sys.modules["antenv.axon_hooks"] = mod
    try:
        import antenv
        antenv.axon_hooks = mod
    except ImportError:
        pass
    try:
        from trn_agent_boot.trn_boot import _ntff_profile_via_ctypes
        mod.set_axon_ntff_profile_hook(
            _ntff_profile_via_ctypes("/opt/axon/libaxon_pjrt.so"))
    except Exception as e:  # degrade to no profiling
        print(f"ntff hook install failed: {e}")


def _run(trace: bool, **inputs):
    if trace:
        _install_ntff_hook()
        import concourse.bass_utils as bu
        bu.upload_artifacts = lambda tmpdir: tmpdir  # no cloud bucket here
    nc = _get_nc()
    in_maps = _prep_in_maps(
        inputs["x"], inputs["Wq"], inputs["bq"], inputs["Wk"], inputs["bk"],
        inputs["Wv"], inputs["bv"])
    res = run_bass_kernel_spmd(nc, in_maps, core_ids=list(range(NCORES)),
                               trace=trace)
    out = np.stack([
        np.asarray(res.results[i]["out"]).reshape(C, H, W)
        for i in range(NCORES)
    ]).astype(np.float32)
    return out, res


def kernel(**inputs) -> np.ndarray:
    out, _ = _run(False, **inputs)
    return out


def kernel_profiled(**inputs):
    out, res = _run(True, **inputs)
    return out, res


# revision 33
# speedup vs baseline: 1.0760x; 1.0760x over previous
"""APNB (asymmetric pyramid non-local block) on 8 TRN2 NeuronCores.

Data-parallel: one batch sample per core. Per core (x: [512, 9216] of one
sample), with all DRAM tensors staged host-side in partition-major layout
(one contiguous run per SBUF partition per DMA -> 128 descriptors/DMA):

  Algebraic restructure 1 (pool/conv commute, both linear):
      psp_pool(conv1x1(x, W, b)) == W @ psp_pool(x) + b
  so the k/v convolutions over the full 96x96 image collapse to tiny
  matmuls on the 110 pooled vectors.

  Algebraic restructure 2 (fold the query conv into the keys; S=110 < O):
      attnT = k_pool.T @ (Wq x + bq) = (Wq.T k_pool).T @ x + (k_pool.T bq) 1^T
  so the full-image query conv disappears; the bias term rides the Exp
  activation as a per-partition bias.

  Pass 1 (streams x + xT from HBM, bf16):
    - pooledT = Mpool.T @ xT       (PE, two alternating PSUM accumulators)
  Finalize:
    - pooled   = pooledT.T         (PE transposes)
    - k_pool   = Wk @ pooled + bk  (PE)
    - v_poolT  = pooled.T @ Wv.T + bv  (PE, rank-1 bias matmul)
    - kq       = Wq.T @ k_pool     (PE)   t = k_pool.T @ bq
  Pass 2 (per 512-column chunk; streams out to HBM):
    - attnT  = kq.T @ x_chunk      [110, 512]  (PE)
    - exp    = exp(attnT + t)      (ACT, psum->sbuf bf16, bias=t)
    - denom  = ones @ exp          (PE, replicated row sums)
    - attn   = exp * approx(1/denom)   (DVE reciprocal_approx_fast + GpSimd mul)
    - out    = v_poolT.T @ attn + x_chunk   (PE; residual via fused DVE add
               for 2 c-tiles and identity-matmul + ACT copy for the other 2)
    - DMA out (fp32, SWDGE)

Softmax needs no max-subtraction: logits are in [-8, 8] for this problem
family (checked against the reference; exp stays finite in fp32).
"""

import numpy as np
import ml_dtypes

import concourse.bass as bass
import concourse.bacc as bacc
import concourse.tile as tile
import concourse.mybir as mybir
from concourse.bass_utils import run_bass_kernel_spmd

BF16 = ml_dtypes.bfloat16
AF = mybir.ActivationFunctionType

B = 8
C = 512          # input/value channels
O = 256          # q/k channels
H = 96
W = 96
N = H * W        # 9216
S = 110          # pooled length 1+9+36+64
PSP = (1, 3, 6, 8)
NCORES = 8
CHUNK = 1024     # columns per input DMA chunk
NBIG = N // CHUNK
SUB = 512        # columns per compute sub-chunk
NSUB = N // SUB  # 18
KT = C // 128    # 4 contraction tiles over channels
MT_O = O // 128  # 2 tiles over q/k channels
NT = N // 128    # 72 position tiles


def _build_pool_matrix() -> np.ndarray:
    m = np.zeros((N, S), dtype=np.float32)
    col = 0
    for s in PSP:
        hb, wb = H // s, W // s
        scale = 1.0 / (hb * wb)
        for i in range(s):
            for j in range(s):
                blk = np.zeros((H, W), np.float32)
                blk[i * hb:(i + 1) * hb, j * wb:(j + 1) * wb] = scale
                m[:, col] = blk.reshape(-1)
                col += 1
    assert col == S
    return m


def _stage(a: np.ndarray) -> np.ndarray:
    """[T*128, F] -> partition-major [128, T*F] (contiguous per partition)."""
    t = a.shape[0] // 128
    return np.ascontiguousarray(
        a.reshape(t, 128, a.shape[1]).transpose(1, 0, 2).reshape(128, -1))


def build_nc() -> bacc.Bacc:
    nc = bacc.Bacc("TRN2", target_bir_lowering=False, debug=False,
                   num_devices=NCORES)
    bf = mybir.dt.bfloat16
    f32 = mybir.dt.float32

    def din(name, shape, dt):
        return nc.dram_tensor(name, shape, dt, kind="ExternalInput").ap()

    # staged layouts: [128, ...] with contiguous free dims as accessed
    x_d = din("x_st", [128, NBIG * KT * CHUNK], bf)     # [p, ci, g, nn]
    xt_d = din("xT_st", [128, NBIG * 8 * C], bf)        # [p, ci, kt, c]
    mp_d = din("mp_st", [128, NT * S], bf)              # [p, t, s]
    wq_d = din("wq_st", [128, MT_O * C], bf)            # [p, t, c]
    wk_d = din("wk_st", [128, KT * O], bf)              # [p, k, m]
    wv_d = din("wv_st", [128, KT * C], bf)              # [p, k, m]
    bq_d = din("bq_col", [O, 1], bf)
    bk_d = din("bk_bf", [1, O], bf)
    bv_d = din("bv_bf", [1, C], bf)
    ones_d = din("ones_bf", [128, 512], bf)
    id_d = din("ident_bf", [128, 128], bf)
    out_d = nc.dram_tensor("out_st", [128, NSUB * KT * SUB], f32,
                           kind="ExternalOutput").ap()   # [p, ci, g, nn]

    xv = x_d.rearrange("p (ci g nn) -> p ci g nn", ci=NBIG, g=KT)
    xtv = xt_d.rearrange("p (ci kt c) -> p ci kt c", ci=NBIG, kt=8)
    mpv = mp_d.rearrange("p (t s) -> p t s", t=NT)
    wqv = wq_d.rearrange("p (t c) -> p t c", t=MT_O)
    wkv = wk_d.rearrange("p (k m) -> p k m", k=KT)
    wvv = wv_d.rearrange("p (k m) -> p k m", k=KT)
    bqv = bq_d.rearrange("(t p) o -> p t o", p=128)
    outv = out_d.rearrange("p (ci g nn) -> p ci g nn", ci=NSUB, g=KT)

    from contextlib import ExitStack
    with tile.TileContext(nc) as tc, ExitStack() as ctx:
        consts = ctx.enter_context(tc.tile_pool(name="consts", bufs=1))
        resid = ctx.enter_context(tc.tile_pool(name="resid", bufs=1))

        # consts via gpsimd SWDGE; ordered so chunk-0 dependencies land first
        mp_sb = consts.tile([128, NT, S], bf)
        nc.gpsimd.dma_start(out=mp_sb, in_=mpv)
        ones_sb = consts.tile([128, 512], bf)
        nc.gpsimd.dma_start(out=ones_sb, in_=ones_d)
        wq_sb = consts.tile([128, MT_O, C], bf)
        nc.gpsimd.dma_start(out=wq_sb, in_=wqv)
        bq_sb = consts.tile([128, MT_O, 1], bf)
        nc.gpsimd.dma_start(out=bq_sb, in_=bqv)
        wk_sb = consts.tile([128, KT, O], bf)
        nc.gpsimd.dma_start(out=wk_sb, in_=wkv)
        wv_sb = consts.tile([128, KT, C], bf)
        nc.gpsimd.dma_start(out=wv_sb, in_=wvv)
        bk_sb = consts.tile([1, O], bf)
        nc.gpsimd.dma_start(out=bk_sb, in_=bk_d)
        bv_sb = consts.tile([1, C], bf)
        nc.gpsimd.dma_start(out=bv_sb, in_=bv_d)
        id_sb = consts.tile([128, 128], bf)
        nc.gpsimd.dma_start(out=id_sb, in_=id_d)

        x_sb = resid.tile([128, NBIG, KT, CHUNK], bf)   # resident input

        k_pool_sb = consts.tile([128, MT_O, S], bf)
        vT_sb = consts.tile([110, C], bf)
        pooledT_sb = consts.tile([110, C], bf)
        pooled_sb = consts.tile([128, KT, S], bf)

        # ------- pass 1: stream x + xT, pool on PE (two alternating chains)
        with tc.tile_pool(name="p1ps", bufs=1, space="PSUM") as p1ps, \
             tc.tile_pool(name="poolps", bufs=1, space="PSUM") as poolps, \
             tc.tile_pool(name="xtp", bufs=3) as xtp:
            pooledT_psa = poolps.tile([110, C], mybir.dt.float32, tag="pa")
            pooledT_psb = poolps.tile([110, C], mybir.dt.float32, tag="pb")
            for ci in range(NBIG):
                xt_t = xtp.tile([128, 8, C], bf, tag="xt")
                nc.gpsimd.dma_start(out=xt_t, in_=xtv[:, ci, :, :])
                nc.sync.dma_start(out=x_sb[:, ci, :, :], in_=xv[:, ci, :, :])
                for kt in range(8):
                    kg = ci * 8 + kt
                    acc = pooledT_psa if kg % 2 == 0 else pooledT_psb
                    nc.tensor.matmul(
                        acc, mp_sb[:, kg, :], xt_t[:, kt, :],
                        start=(kg < 2), stop=(kg >= NT - 2),
                        skip_group_check=True)

            # ---------------- finalize pools ----------------
            ptmp_sb = consts.tile([110, C], mybir.dt.float32)
            nc.scalar.copy(ptmp_sb, pooledT_psa)
            with nc.allow_low_precision("pooled stats stored bf16"):
                nc.vector.tensor_add(pooledT_sb, pooledT_psb, ptmp_sb)
            for c in range(KT):
                tr_ps = p1ps.tile([128, S], mybir.dt.bfloat16, tag="trps")
                nc.tensor.transpose(
                    tr_ps, pooledT_sb[:, c * 128:(c + 1) * 128],
                    id_sb[:110, :110])
                nc.scalar.copy(pooled_sb[:, c, :], tr_ps)
            kp_ps = p1ps.tile([128, MT_O, 128], mybir.dt.float32, tag="kps")
            for m in range(MT_O):
                for k in range(KT):
                    nc.tensor.matmul(
                        kp_ps[:, m, 0:S], wk_sb[:, k, m * 128:(m + 1) * 128],
                        pooled_sb[:, k, :], start=(k == 0), stop=False,
                        skip_group_check=True)
                nc.tensor.matmul(
                    kp_ps[:, m, 0:S], bk_sb[0:1, m * 128:(m + 1) * 128],
                    ones_sb[0:1, 0:S], start=False, stop=True,
                    skip_group_check=True)
            nc.scalar.copy(k_pool_sb, kp_ps[:, :, 0:S])
            # attnT = (Wq.T k_pool).T @ x + (k_pool.T bq) 1^T
            t_ps = p1ps.tile([110, 128], mybir.dt.float32, tag="tps")
            for kt in range(MT_O):
                nc.tensor.matmul(t_ps[:, 0:1], k_pool_sb[:, kt, :],
                                 bq_sb[:, kt, :],
                                 start=(kt == 0), stop=(kt == MT_O - 1),
                                 skip_group_check=True)
            t_sb = consts.tile([110, 1], mybir.dt.float32)
            nc.scalar.copy(t_sb, t_ps[:, 0:1])
            kq_sb = consts.tile([128, KT, S], bf)
            for cb in range(KT):
                kq_ps = p1ps.tile([128, 128], mybir.dt.float32, tag="tps")
                for kt in range(MT_O):
                    nc.tensor.matmul(
                        kq_ps[:, 0:S],
                        wq_sb[:, kt, cb * 128:(cb + 1) * 128],
                        k_pool_sb[:, kt, :],
                        start=(kt == 0), stop=(kt == MT_O - 1),
                        skip_group_check=True)
                nc.scalar.copy(kq_sb[:, cb, :], kq_ps[:, 0:S])
            vp_ps = p1ps.tile([110, C], mybir.dt.float32, tag="vps")
            for k in range(KT):
                nc.tensor.matmul(vp_ps, pooled_sb[:, k, :], wv_sb[:, k, :],
                                 start=(k == 0), stop=False,
                                 skip_group_check=True)
            nc.tensor.matmul(vp_ps, ones_sb[0:1, :110], bv_sb,
                             start=False, stop=True, skip_group_check=True)
            nc.scalar.copy(vT_sb, vp_ps)

        # ------- pass 2: folded attention + output, per 512-col chunk ----
        with tc.tile_pool(name="p2ps", bufs=2, space="PSUM") as p2ps, \
             tc.tile_pool(name="pops", bufs=3, space="PSUM") as pops, \
             tc.tile_pool(name="p2sb", bufs=3) as p2sb, \
             tc.tile_pool(name="outp", bufs=4) as outp:
            for ci in range(NSUB):
                cb, sub = divmod(ci, CHUNK // SUB)
                lsl = slice(sub * SUB, (sub + 1) * SUB)
                at_ps = p2ps.tile([110, SUB], mybir.dt.float32, tag="atdb")
                for k in range(KT):
                    nc.tensor.matmul(at_ps, kq_sb[:, k, :],
                                     x_sb[:, cb, k, lsl],
                                     start=(k == 0), stop=(k == KT - 1))
                exp_sb = p2sb.tile([110, SUB], mybir.dt.bfloat16, tag="exp")
                nc.scalar.activation(exp_sb, at_ps, AF.Exp,
                                     bias=t_sb, scale=1.0)
                db_ps = p2ps.tile([110, SUB], mybir.dt.float32, tag="atdb")
                nc.tensor.matmul(db_ps, ones_sb[:110, :110], exp_sb,
                                 start=True, stop=True)
                recip_sb = p2sb.tile([110, SUB], mybir.dt.float32, tag="recip")
                nc.vector.reciprocal_approx_fast(recip_sb, db_ps)
                attn_sb = p2sb.tile([110, SUB], mybir.dt.bfloat16, tag="attn")
                with nc.allow_low_precision("softmax weights tolerate bf16"):
                    nc.gpsimd.tensor_mul(attn_sb, exp_sb, recip_sb)
                out_t = outp.tile([128, KT, SUB], mybir.dt.float32, tag="out")
                # c 0,1: residual fused into the psum->sbuf move on DVE
                o_psa = pops.tile([128, 2, SUB], mybir.dt.float32, tag="ops")
                for c in range(2):
                    nc.tensor.matmul(o_psa[:, c, :],
                                     vT_sb[:, c * 128:(c + 1) * 128],
                                     attn_sb, start=True, stop=True,
                                     skip_group_check=True)
                with nc.allow_low_precision("fp32 psum + bf16 residual"):
                    nc.vector.tensor_add(out_t[:, 0:2, :], o_psa,
                                         x_sb[:, cb, 0:2, lsl])
                # c 2,3: residual via identity matmul, copy on ACT
                o_psb = pops.tile([128, 2, SUB], mybir.dt.float32, tag="ops")
                for c in range(2, KT):
                    nc.tensor.matmul(o_psb[:, c - 2, :],
                                     vT_sb[:, c * 128:(c + 1) * 128],
                                     attn_sb, start=True, stop=False,
                                     skip_group_check=True)
                    nc.tensor.matmul(o_psb[:, c - 2, :], id_sb,
                                     x_sb[:, cb, c, lsl],
                                     start=False, stop=True,
                                     skip_group_check=True)
                nc.scalar.copy(out_t[:, 2:4, :], o_psb)
                nc.gpsimd.dma_start(out=outv[:, ci, :, :], in_=out_t)

    nc.compile()
    return nc


_NC_CACHE = None


def _get_nc() -> bacc.Bacc:
    global _NC_CACHE
    if _NC_CACHE is None:
        _NC_CACHE = build_nc()
    return _NC_CACHE


def _prep_in_maps(x, Wq, bq, Wk, bk, Wv, bv):
    shared = {
        "mp_st": _stage(_build_pool_matrix().astype(BF16)),
        "wq_st": _stage(np.ascontiguousarray(Wq).astype(BF16)),
        "wk_st": _stage(np.ascontiguousarray(Wk.T).astype(BF16)),
        "wv_st": _stage(np.ascontiguousarray(Wv.T).astype(BF16)),
        "bq_col": np.ascontiguousarray(bq.reshape(O, 1)).astype(BF16),
        "bk_bf": np.ascontiguousarray(bk.reshape(1, O)).astype(BF16),
        "bv_bf": np.ascontiguousarray(bv.reshape(1, C)).astype(BF16),
        "ones_bf": np.ones((128, 512), dtype=BF16),
        "ident_bf": np.eye(128, dtype=np.float32).astype(BF16),
    }
    in_maps = []
    for i in range(NCORES):
        xi = np.ascontiguousarray(x[i].reshape(C, N))
        xi_bf = xi.astype(BF16)
        # x: [p, ci, g, nn] with g the 128-channel block
        x_st = np.ascontiguousarray(
            xi_bf.reshape(KT, 128, NBIG, CHUNK).transpose(1, 2, 0, 3)
            .reshape(128, -1))
        # xT: [p, ci, kt, c]
        xT_st = np.ascontiguousarray(
            xi_bf.T.reshape(NBIG, 8, 128, C).transpose(2, 0, 1, 3)
            .reshape(128, -1))
        m = dict(shared)
        m["x_st"] = x_st
        m["xT_st"] = xT_st
        in_maps.append(m)
    return in_maps


def _unstage_out(o: np.ndarray) -> np.ndarray:
    # [128, NSUB*KT*SUB] -> [C, H, W]
    return np.ascontiguousarray(
        o.reshape(128, NSUB, KT, SUB).transpose(2, 0, 1, 3)
        .reshape(C, N)).reshape(C, H, W)


def _install_ntff_hook():
    """The agent image ships no antenv.axon_hooks module, so trace=True
    under axon crashes on import. Recreate the tiny hook-holder module and
    register trn_boot's ctypes NTFF hook so neuron-profile timing works."""
    import sys
    import types
    if "antenv.axon_hooks" in sys.modules:
        return
    mod = types.ModuleType("antenv.axon_hooks")
    holder = {"h": None}
    mod.set_axon_ntff_profile_hook = lambda h: holder.__setitem__("h", h)
    mod.get_axon_ntff_profile_hook = lambda: holder["h"]
    sys.modules["antenv.axon_hooks"] = mod
    try:
        import antenv
        antenv.axon_hooks = mod
    except ImportError:
        pass
    try:
        from trn_agent_boot.trn_boot import _ntff_profile_via_ctypes
        mod.set_axon_ntff_profile_hook(
            _ntff_profile_via_ctypes("/opt/axon/libaxon_pjrt.so"))
    except Exception as e:  # degrade to no profiling
        print(f"ntff hook install failed: {e}")


def _run(trace: bool, **inputs):
    if trace:
        _install_ntff_hook()
        import concourse.bass_utils as bu
        bu.upload_artifacts = lambda tmpdir: tmpdir  # no cloud bucket here
    nc = _get_nc()
    in_maps = _prep_in_maps(
        inputs["x"], inputs["Wq"], inputs["bq"], inputs["Wk"], inputs["bk"],
        inputs["Wv"], inputs["bv"])
    res = run_bass_kernel_spmd(nc, in_maps, core_ids=list(range(NCORES)),
                               trace=trace)
    out = np.stack([
        _unstage_out(np.asarray(res.results[i]["out_st"]))
        for i in range(NCORES)
    ]).astype(np.float32)
    return out, res


def kernel(**inputs) -> np.ndarray:
    out, _ = _run(False, **inputs)
    return out


def kernel_profiled(**inputs):
    out, res = _run(True, **inputs)
    return out, res


# revision 34
# speedup vs baseline: 1.2367x; 1.1494x over previous
"""APNB (asymmetric pyramid non-local block) on 8 TRN2 NeuronCores.

Data-parallel: one batch sample per core. Per core (x: [512, 9216] of one
sample), with all DRAM tensors staged host-side in partition-major layout
(one contiguous run per SBUF partition per DMA -> 128 descriptors/DMA):

  Algebraic restructure 1 (pool/conv commute, both linear):
      psp_pool(conv1x1(x, W, b)) == W @ psp_pool(x) + b
  so the k/v convolutions over the full 96x96 image collapse to tiny
  matmuls on the 110 pooled vectors.

  Algebraic restructure 2 (fold the query conv into the keys; S=110 < O):
      attnT = k_pool.T @ (Wq x + bq) = (Wq.T k_pool).T @ x + (k_pool.T bq) 1^T
  so the full-image query conv disappears; the bias term rides the Exp
  activation as a per-partition bias.

  Pass 1 (streams x + xT from HBM, bf16):
    - pooledT = Mpool.T @ xT       (PE, two alternating PSUM accumulators)
  Finalize:
    - pooled   = pooledT.T         (PE transposes)
    - k_pool   = Wk @ pooled + bk  (PE)
    - v_poolT  = pooled.T @ Wv.T + bv  (PE, rank-1 bias matmul)
    - kq       = Wq.T @ k_pool     (PE)   t = k_pool.T @ bq
  Pass 2 (per 512-column chunk; streams out to HBM):
    - attnT  = kq.T @ x_chunk      [110, 512]  (PE)
    - exp    = exp(attnT + t)      (ACT, psum->sbuf bf16, bias=t)
    - denom  = ones @ exp          (PE, replicated row sums)
    - attn   = exp * approx(1/denom)   (DVE reciprocal_approx_fast + GpSimd mul)
    - out    = v_poolT.T @ attn + x_chunk   (PE; residual via fused DVE add
               for 2 c-tiles and identity-matmul + ACT copy for the other 2)
    - DMA out (fp32, SWDGE)

Softmax needs no max-subtraction: logits are in [-8, 8] for this problem
family (checked against the reference; exp stays finite in fp32).
"""

import numpy as np
import ml_dtypes

import concourse.bass as bass
import concourse.bacc as bacc
import concourse.tile as tile
import concourse.mybir as mybir
from concourse.bass_utils import run_bass_kernel_spmd

BF16 = ml_dtypes.bfloat16
AF = mybir.ActivationFunctionType

B = 8
C = 512          # input/value channels
O = 256          # q/k channels
H = 96
W = 96
N = H * W        # 9216
S = 110          # pooled length 1+9+36+64
PSP = (1, 3, 6, 8)
NCORES = 8
CHUNK = 1024     # columns per input DMA chunk
NBIG = N // CHUNK
SUB = 512        # columns per compute sub-chunk
NSUB = N // SUB  # 18
KT = C // 128    # 4 contraction tiles over channels
MT_O = O // 128  # 2 tiles over q/k channels
NT = N // 128    # 72 position tiles


def _build_pool_matrix() -> np.ndarray:
    m = np.zeros((N, S), dtype=np.float32)
    col = 0
    for s in PSP:
        hb, wb = H // s, W // s
        scale = 1.0 / (hb * wb)
        for i in range(s):
            for j in range(s):
                blk = np.zeros((H, W), np.float32)
                blk[i * hb:(i + 1) * hb, j * wb:(j + 1) * wb] = scale
                m[:, col] = blk.reshape(-1)
                col += 1
    assert col == S
    return m


def _stage(a: np.ndarray) -> np.ndarray:
    """[T*128, F] -> partition-major [128, T*F] (contiguous per partition)."""
    t = a.shape[0] // 128
    return np.ascontiguousarray(
        a.reshape(t, 128, a.shape[1]).transpose(1, 0, 2).reshape(128, -1))


def build_nc() -> bacc.Bacc:
    nc = bacc.Bacc("TRN2", target_bir_lowering=False, debug=False,
                   num_devices=NCORES)
    bf = mybir.dt.bfloat16
    f32 = mybir.dt.float32

    def din(name, shape, dt):
        return nc.dram_tensor(name, shape, dt, kind="ExternalInput").ap()

    # staged layouts: [128, ...] with contiguous free dims as accessed
    x_d = din("x_st", [128, NBIG * KT * CHUNK], bf)     # [p, ci, g, nn]
    xt_d = din("xT_st", [128, NBIG * 8 * C], bf)        # [p, ci, kt, c]
    mp_d = din("mp_st", [128, NT * S], bf)              # [p, t, s]
    wq_d = din("wq_st", [128, MT_O * C], bf)            # [p, t, c]
    wk_d = din("wk_st", [128, KT * O], bf)              # [p, k, m]
    wv_d = din("wv_st", [128, KT * C], bf)              # [p, k, m]
    bq_d = din("bq_col", [O, 1], bf)
    bk_d = din("bk_bf", [1, O], bf)
    bv_d = din("bv_bf", [1, C], bf)
    ones_d = din("ones_bf", [128, 512], bf)
    id_d = din("ident_bf", [128, 128], bf)
    out_d = nc.dram_tensor("out_st", [128, NSUB * KT * SUB], f32,
                           kind="ExternalOutput").ap()   # [p, ci, g, nn]

    xv = x_d.rearrange("p (ci g nn) -> p ci g nn", ci=NBIG, g=KT)
    xtv = xt_d.rearrange("p (ci kt c) -> p ci kt c", ci=NBIG, kt=8)
    mpv = mp_d.rearrange("p (t s) -> p t s", t=NT)
    wqv = wq_d.rearrange("p (t c) -> p t c", t=MT_O)
    wkv = wk_d.rearrange("p (k m) -> p k m", k=KT)
    wvv = wv_d.rearrange("p (k m) -> p k m", k=KT)
    bqv = bq_d.rearrange("(t p) o -> p t o", p=128)
    outv = out_d.rearrange("p (ci g nn) -> p ci g nn", ci=NSUB, g=KT)

    from contextlib import ExitStack
    with tile.TileContext(nc) as tc, ExitStack() as ctx:
        consts = ctx.enter_context(tc.tile_pool(name="consts", bufs=1))
        resid = ctx.enter_context(tc.tile_pool(name="resid", bufs=1))

        # consts via gpsimd SWDGE; ordered so chunk-0 dependencies land first
        mp_sb = consts.tile([128, NT, S], bf)
        nc.gpsimd.dma_start(out=mp_sb, in_=mpv)
        ones_sb = consts.tile([128, 512], bf)
        nc.gpsimd.dma_start(out=ones_sb, in_=ones_d)
        wq_sb = consts.tile([128, MT_O, C], bf)
        nc.gpsimd.dma_start(out=wq_sb, in_=wqv)
        bq_sb = consts.tile([128, MT_O, 1], bf)
        nc.gpsimd.dma_start(out=bq_sb, in_=bqv)
        wk_sb = consts.tile([128, KT, O], bf)
        nc.gpsimd.dma_start(out=wk_sb, in_=wkv)
        wv_sb = consts.tile([128, KT, C], bf)
        nc.gpsimd.dma_start(out=wv_sb, in_=wvv)
        bk_sb = consts.tile([1, O], bf)
        nc.gpsimd.dma_start(out=bk_sb, in_=bk_d)
        bv_sb = consts.tile([1, C], bf)
        nc.gpsimd.dma_start(out=bv_sb, in_=bv_d)
        id_sb = consts.tile([128, 128], bf)
        nc.gpsimd.dma_start(out=id_sb, in_=id_d)

        x_sb = resid.tile([128, NBIG, KT, CHUNK], bf)   # resident input

        k_pool_sb = consts.tile([128, MT_O, S], bf)
        vT_sb = consts.tile([110, C], bf)
        pooledT_sb = consts.tile([110, C], bf)
        pooled_sb = consts.tile([128, KT, S], bf)

        # ------- pass 1: stream x + xT, pool on PE (two alternating chains)
        with tc.tile_pool(name="p1ps", bufs=1, space="PSUM") as p1ps, \
             tc.tile_pool(name="poolps", bufs=1, space="PSUM") as poolps, \
             tc.tile_pool(name="xtp", bufs=3) as xtp:
            pooledT_psa = poolps.tile([110, C], mybir.dt.float32, tag="pa")
            pooledT_psb = poolps.tile([110, C], mybir.dt.float32, tag="pb")
            for ci in range(NBIG):
                xt_t = xtp.tile([128, 8, C], bf, tag="xt")
                nc.gpsimd.dma_start(out=xt_t, in_=xtv[:, ci, :, :])
                for kt in range(8):
                    kg = ci * 8 + kt
                    acc = pooledT_psa if kg % 2 == 0 else pooledT_psb
                    nc.tensor.matmul(
                        acc, mp_sb[:, kg, :], xt_t[:, kt, :],
                        start=(kg < 2), stop=(kg >= NT - 2),
                        skip_group_check=True)
            # x stream rides the same SWDGE queue AFTER the xT stream, so
            # pooling (and the finalize it gates) isn't starved by x traffic;
            # x arrivals comfortably lead pass-2 consumption.
            for ci in range(NBIG):
                nc.gpsimd.dma_start(out=x_sb[:, ci, :, :], in_=xv[:, ci, :, :])

            # ---------------- finalize pools ----------------
            ptmp_sb = consts.tile([110, C], mybir.dt.float32)
            nc.scalar.copy(ptmp_sb, pooledT_psa)
            with nc.allow_low_precision("pooled stats stored bf16"):
                nc.vector.tensor_add(pooledT_sb, pooledT_psb, ptmp_sb)
            for c in range(KT):
                tr_ps = p1ps.tile([128, S], mybir.dt.bfloat16, tag="trps")
                nc.tensor.transpose(
                    tr_ps, pooledT_sb[:, c * 128:(c + 1) * 128],
                    id_sb[:110, :110])
                nc.scalar.copy(pooled_sb[:, c, :], tr_ps)
            kp_ps = p1ps.tile([128, MT_O, 128], mybir.dt.float32, tag="kps")
            for m in range(MT_O):
                for k in range(KT):
                    nc.tensor.matmul(
                        kp_ps[:, m, 0:S], wk_sb[:, k, m * 128:(m + 1) * 128],
                        pooled_sb[:, k, :], start=(k == 0), stop=False,
                        skip_group_check=True)
                nc.tensor.matmul(
                    kp_ps[:, m, 0:S], bk_sb[0:1, m * 128:(m + 1) * 128],
                    ones_sb[0:1, 0:S], start=False, stop=True,
                    skip_group_check=True)
            nc.scalar.copy(k_pool_sb, kp_ps[:, :, 0:S])
            # attnT = (Wq.T k_pool).T @ x + (k_pool.T bq) 1^T
            t_ps = p1ps.tile([110, 128], mybir.dt.float32, tag="tps")
            for kt in range(MT_O):
                nc.tensor.matmul(t_ps[:, 0:1], k_pool_sb[:, kt, :],
                                 bq_sb[:, kt, :],
                                 start=(kt == 0), stop=(kt == MT_O - 1),
                                 skip_group_check=True)
            t_sb = consts.tile([110, 1], mybir.dt.float32)
            nc.scalar.copy(t_sb, t_ps[:, 0:1])
            kq_sb = consts.tile([128, KT, S], bf)
            for cb in range(KT):
                kq_ps = p1ps.tile([128, 128], mybir.dt.float32, tag="tps")
                for kt in range(MT_O):
                    nc.tensor.matmul(
                        kq_ps[:, 0:S],
                        wq_sb[:, kt, cb * 128:(cb + 1) * 128],
                        k_pool_sb[:, kt, :],
                        start=(kt == 0), stop=(kt == MT_O - 1),
                        skip_group_check=True)
                nc.scalar.copy(kq_sb[:, cb, :], kq_ps[:, 0:S])
            vp_ps = p1ps.tile([110, C], mybir.dt.float32, tag="vps")
            for k in range(KT):
                nc.tensor.matmul(vp_ps, pooled_sb[:, k, :], wv_sb[:, k, :],
                                 start=(k == 0), stop=False,
                                 skip_group_check=True)
            nc.tensor.matmul(vp_ps, ones_sb[0:1, :110], bv_sb,
                             start=False, stop=True, skip_group_check=True)
            nc.scalar.copy(vT_sb, vp_ps)

        # ------- pass 2: folded attention + output, per 512-col chunk ----
        with tc.tile_pool(name="p2ps", bufs=2, space="PSUM") as p2ps, \
             tc.tile_pool(name="pops", bufs=3, space="PSUM") as pops, \
             tc.tile_pool(name="p2sb", bufs=3) as p2sb, \
             tc.tile_pool(name="outp", bufs=4) as outp:
            for ci in range(NSUB):
                cb, sub = divmod(ci, CHUNK // SUB)
                lsl = slice(sub * SUB, (sub + 1) * SUB)
                at_ps = p2ps.tile([110, SUB], mybir.dt.float32, tag="atdb")
                for k in range(KT):
                    nc.tensor.matmul(at_ps, kq_sb[:, k, :],
                                     x_sb[:, cb, k, lsl],
                                     start=(k == 0), stop=(k == KT - 1))
                exp_sb = p2sb.tile([110, SUB], mybir.dt.bfloat16, tag="exp")
                nc.scalar.activation(exp_sb, at_ps, AF.Exp,
                                     bias=t_sb, scale=1.0)
                db_ps = p2ps.tile([110, SUB], mybir.dt.float32, tag="atdb")
                nc.tensor.matmul(db_ps, ones_sb[:110, :110], exp_sb,
                                 start=True, stop=True)
                recip_sb = p2sb.tile([110, SUB], mybir.dt.float32, tag="recip")
                nc.vector.reciprocal_approx_fast(recip_sb, db_ps)
                attn_sb = p2sb.tile([110, SUB], mybir.dt.bfloat16, tag="attn")
                with nc.allow_low_precision("softmax weights tolerate bf16"):
                    nc.gpsimd.tensor_mul(attn_sb, exp_sb, recip_sb)
                out_t = outp.tile([128, KT, SUB], mybir.dt.float32, tag="out")
                # c 0,1: residual fused into the psum->sbuf move on DVE
                o_psa = pops.tile([128, 2, SUB], mybir.dt.float32, tag="ops")
                for c in range(2):
                    nc.tensor.matmul(o_psa[:, c, :],
                                     vT_sb[:, c * 128:(c + 1) * 128],
                                     attn_sb, start=True, stop=True,
                                     skip_group_check=True)
                with nc.allow_low_precision("fp32 psum + bf16 residual"):
                    nc.vector.tensor_add(out_t[:, 0:2, :], o_psa,
                                         x_sb[:, cb, 0:2, lsl])
                # c 2,3: residual via identity matmul, copy on ACT
                o_psb = pops.tile([128, 2, SUB], mybir.dt.float32, tag="ops")
                for c in range(2, KT):
                    nc.tensor.matmul(o_psb[:, c - 2, :],
                                     vT_sb[:, c * 128:(c + 1) * 128],
                                     attn_sb, start=True, stop=False,
                                     skip_group_check=True)
                    nc.tensor.matmul(o_psb[:, c - 2, :], id_sb,
                                     x_sb[:, cb, c, lsl],
                                     start=False, stop=True,
                                     skip_group_check=True)
                nc.scalar.copy(out_t[:, 2:4, :], o_psb)
                nc.gpsimd.dma_start(out=outv[:, ci, :, :], in_=out_t)

    nc.compile()
    return nc


_NC_CACHE = None


def _get_nc() -> bacc.Bacc:
    global _NC_CACHE
    if _NC_CACHE is None:
        _NC_CACHE = build_nc()
    return _NC_CACHE


def _prep_in_maps(x, Wq, bq, Wk, bk, Wv, bv):
    shared = {
        "mp_st": _stage(_build_pool_matrix().astype(BF16)),
        "wq_st": _stage(np.ascontiguousarray(Wq).astype(BF16)),
        "wk_st": _stage(np.ascontiguousarray(Wk.T).astype(BF16)),
        "wv_st": _stage(np.ascontiguousarray(Wv.T).astype(BF16)),
        "bq_col": np.ascontiguousarray(bq.reshape(O, 1)).astype(BF16),
        "bk_bf": np.ascontiguousarray(bk.reshape(1, O)).astype(BF16),
        "bv_bf": np.ascontiguousarray(bv.reshape(1, C)).astype(BF16),
        "ones_bf": np.ones((128, 512), dtype=BF16),
        "ident_bf": np.eye(128, dtype=np.float32).astype(BF16),
    }
    in_maps = []
    for i in range(NCORES):
        xi = np.ascontiguousarray(x[i].reshape(C, N))
        xi_bf = xi.astype(BF16)
        # x: [p, ci, g, nn] with g the 128-channel block
        x_st = np.ascontiguousarray(
            xi_bf.reshape(KT, 128, NBIG, CHUNK).transpose(1, 2, 0, 3)
            .reshape(128, -1))
        # xT: [p, ci, kt, c]
        xT_st = np.ascontiguousarray(
            xi_bf.T.reshape(NBIG, 8, 128, C).transpose(2, 0, 1, 3)
            .reshape(128, -1))
        m = dict(shared)
        m["x_st"] = x_st
        m["xT_st"] = xT_st
        in_maps.append(m)
    return in_maps


def _unstage_out(o: np.ndarray) -> np.ndarray:
    # [128, NSUB*KT*SUB] -> [C, H, W]
    return np.ascontiguousarray(
        o.reshape(128, NSUB, KT, SUB).transpose(2, 0, 1, 3)
        .reshape(C, N)).reshape(C, H, W)


def _install_ntff_hook():
    """The agent image ships no antenv.axon_hooks module, so trace=True
    under axon crashes on import. Recreate the tiny hook-holder module and
    register trn_boot's ctypes NTFF hook so neuron-profile timing works."""
    import sys
    import types
    if "antenv.axon_hooks" in sys.modules:
        return
    mod = types.ModuleType("antenv.axon_hooks")
    holder = {"h": None}
    mod.set_axon_ntff_profile_hook = lambda h: holder.__setitem__("h", h)
    mod.get_axon_ntff_profile_hook = lambda: holder["h"]
    sys.modules["antenv.axon_hooks"] = mod
    try:
        import antenv
        antenv.axon_hooks = mod
    except ImportError:
        pass
    try:
        from trn_agent_boot.trn_boot import _ntff_profile_via_ctypes
        mod.set_axon_ntff_profile_hook(
            _ntff_profile_via_ctypes("/opt/axon/libaxon_pjrt.so"))
    except Exception as e:  # degrade to no profiling
        print(f"ntff hook install failed: {e}")


def _run(trace: bool, **inputs):
    if trace:
        _install_ntff_hook()
        import concourse.bass_utils as bu
        bu.upload_artifacts = lambda tmpdir: tmpdir  # no cloud bucket here
    nc = _get_nc()
    in_maps = _prep_in_maps(
        inputs["x"], inputs["Wq"], inputs["bq"], inputs["Wk"], inputs["bk"],
        inputs["Wv"], inputs["bv"])
    res = run_bass_kernel_spmd(nc, in_maps, core_ids=list(range(NCORES)),
                               trace=trace)
    out = np.stack([
        _unstage_out(np.asarray(res.results[i]["out_st"]))
        for i in range(NCORES)
    ]).astype(np.float32)
    return out, res


def kernel(**inputs) -> np.ndarray:
    out, _ = _run(False, **inputs)
    return out


def kernel_profiled(**inputs):
    out, res = _run(True, **inputs)
    return out, res


# revision 36
# speedup vs baseline: 1.3077x; 1.0574x over previous
"""APNB (asymmetric pyramid non-local block) on 8 TRN2 NeuronCores.

Data-parallel: one batch sample per core. Per core (x: [512, 9216] of one
sample), with all DRAM tensors staged host-side in partition-major layout
(one contiguous run per SBUF partition per DMA -> 128 descriptors/DMA):

  Algebraic restructure 1 (pool/conv commute, both linear):
      psp_pool(conv1x1(x, W, b)) == W @ psp_pool(x) + b
  so the k/v convolutions over the full 96x96 image collapse to tiny
  matmuls on the 110 pooled vectors.

  Algebraic restructure 2 (fold the query conv into the keys; S=110 < O):
      attnT = k_pool.T @ (Wq x + bq) = (Wq.T k_pool).T @ x + (k_pool.T bq) 1^T
  so the full-image query conv disappears; the bias term rides the Exp
  activation as a per-partition bias.

  Pass 1 (streams x + xT from HBM, bf16):
    - pooledT = Mpool.T @ xT       (PE, two alternating PSUM accumulators)
  Finalize:
    - pooled   = pooledT.T         (PE transposes)
    - k_pool   = Wk @ pooled + bk  (PE)
    - v_poolT  = pooled.T @ Wv.T + bv  (PE, rank-1 bias matmul)
    - kq       = Wq.T @ k_pool     (PE)   t = k_pool.T @ bq
  Pass 2 (per 512-column chunk; streams out to HBM):
    - attnT  = kq.T @ x_chunk      [110, 512]  (PE)
    - exp    = exp(attnT + t)      (ACT, psum->sbuf bf16, bias=t)
    - denom  = ones @ exp          (PE, replicated row sums)
    - attn   = exp * approx(1/denom)   (DVE reciprocal_approx_fast + GpSimd mul)
    - out    = v_poolT.T @ attn + x_chunk   (PE; residual via fused DVE add
               for 2 c-tiles and identity-matmul + ACT copy for the other 2)
    - DMA out (fp32, SWDGE)

Softmax needs no max-subtraction: logits are in [-8, 8] for this problem
family (checked against the reference; exp stays finite in fp32).
"""

import numpy as np
import ml_dtypes

import concourse.bass as bass
import concourse.bacc as bacc
import concourse.tile as tile
import concourse.mybir as mybir
from concourse.bass_utils import run_bass_kernel_spmd

BF16 = ml_dtypes.bfloat16
AF = mybir.ActivationFunctionType

B = 8
C = 512          # input/value channels
O = 256          # q/k channels
H = 96
W = 96
N = H * W        # 9216
S = 110          # pooled length 1+9+36+64
PSP = (1, 3, 6, 8)
NCORES = 8
CHUNK = 1024     # columns per input DMA chunk
NBIG = N // CHUNK
SUB = 512        # columns per compute sub-chunk
NSUB = N // SUB  # 18
KT = C // 128    # 4 contraction tiles over channels
MT_O = O // 128  # 2 tiles over q/k channels
NT = N // 128    # 72 position tiles


def _build_pool_matrix() -> np.ndarray:
    m = np.zeros((N, S), dtype=np.float32)
    col = 0
    for s in PSP:
        hb, wb = H // s, W // s
        scale = 1.0 / (hb * wb)
        for i in range(s):
            for j in range(s):
                blk = np.zeros((H, W), np.float32)
                blk[i * hb:(i + 1) * hb, j * wb:(j + 1) * wb] = scale
                m[:, col] = blk.reshape(-1)
                col += 1
    assert col == S
    return m


def _stage(a: np.ndarray) -> np.ndarray:
    """[T*128, F] -> partition-major [128, T*F] (contiguous per partition)."""
    t = a.shape[0] // 128
    return np.ascontiguousarray(
        a.reshape(t, 128, a.shape[1]).transpose(1, 0, 2).reshape(128, -1))


def build_nc() -> bacc.Bacc:
    nc = bacc.Bacc("TRN2", target_bir_lowering=False, debug=False,
                   num_devices=NCORES)
    bf = mybir.dt.bfloat16
    f32 = mybir.dt.float32

    def din(name, shape, dt):
        return nc.dram_tensor(name, shape, dt, kind="ExternalInput").ap()

    # staged layouts: [128, ...] with contiguous free dims as accessed
    x_d = din("x_st", [128, NBIG * KT * CHUNK], bf)     # [p, ci, g, nn]
    xt_d = din("xT_st", [128, NBIG * 8 * C], bf)        # [p, ci, kt, c]
    mp_d = din("mp_st", [128, NT * S], bf)              # [p, t, s]
    wq_d = din("wq_st", [128, MT_O * C], bf)            # [p, t, c]
    wk_d = din("wk_st", [128, KT * O], bf)              # [p, k, m]
    wv_d = din("wv_st", [128, KT * C], bf)              # [p, k, m]
    bq_d = din("bq_col", [O, 1], bf)
    bk_d = din("bk_bf", [1, O], bf)
    bv_d = din("bv_bf", [1, C], bf)
    ones_d = din("ones_bf", [128, 512], bf)
    id_d = din("ident_bf", [128, 128], bf)
    out_d = nc.dram_tensor("out_st", [128, NSUB * KT * SUB], f32,
                           kind="ExternalOutput").ap()   # [p, ci, g, nn]

    xv = x_d.rearrange("p (ci g nn) -> p ci g nn", ci=NBIG, g=KT)
    xtv = xt_d.rearrange("p (ci kt c) -> p ci kt c", ci=NBIG, kt=8)
    mpv = mp_d.rearrange("p (t s) -> p t s", t=NT)
    wqv = wq_d.rearrange("p (t c) -> p t c", t=MT_O)
    wkv = wk_d.rearrange("p (k m) -> p k m", k=KT)
    wvv = wv_d.rearrange("p (k m) -> p k m", k=KT)
    bqv = bq_d.rearrange("(t p) o -> p t o", p=128)
    outv = out_d.rearrange("p (ci g nn) -> p ci g nn", ci=NSUB, g=KT)

    from contextlib import ExitStack
    with tile.TileContext(nc) as tc, ExitStack() as ctx:
        consts = ctx.enter_context(tc.tile_pool(name="consts", bufs=1))
        resid = ctx.enter_context(tc.tile_pool(name="resid", bufs=1))

        # small consts via gpsimd SWDGE, ahead of the xT stream; mpool itself
        # is split per-chunk inside the pass-1 loop so pooling starts at once
        mp_sb = consts.tile([128, NT, S], bf)
        id_sb = consts.tile([128, 128], bf)
        nc.gpsimd.dma_start(out=id_sb, in_=id_d)
        wk_sb = consts.tile([128, KT, O], bf)
        nc.gpsimd.dma_start(out=wk_sb, in_=wkv)
        ones_sb = consts.tile([128, 512], bf)
        nc.gpsimd.dma_start(out=ones_sb, in_=ones_d)
        wq_sb = consts.tile([128, MT_O, C], bf)
        nc.gpsimd.dma_start(out=wq_sb, in_=wqv)
        bq_sb = consts.tile([128, MT_O, 1], bf)
        nc.gpsimd.dma_start(out=bq_sb, in_=bqv)

        x_sb = resid.tile([128, NBIG, KT, CHUNK], bf)   # resident input

        k_pool_sb = consts.tile([128, MT_O, S], bf)
        vT_sb = consts.tile([110, C], bf)
        pooledT_sb = consts.tile([110, C], bf)
        pooled_sb = consts.tile([128, KT, S], bf)

        # ------- pass 1: stream x + xT, pool on PE (two alternating chains)
        with tc.tile_pool(name="p1ps", bufs=1, space="PSUM") as p1ps, \
             tc.tile_pool(name="poolps", bufs=1, space="PSUM") as poolps, \
             tc.tile_pool(name="xtp", bufs=3) as xtp:
            pooledT_ps = poolps.tile([110, C], mybir.dt.float32, tag="pa")
            for ci in range(NBIG):
                nc.gpsimd.dma_start(out=mp_sb[:, ci * 8:(ci + 1) * 8, :],
                                    in_=mpv[:, ci * 8:(ci + 1) * 8, :])
                xt_t = xtp.tile([128, 8, C], bf, tag="xt")
                nc.gpsimd.dma_start(out=xt_t, in_=xtv[:, ci, :, :])
                for kt in range(8):
                    kg = ci * 8 + kt
                    nc.tensor.matmul(
                        pooledT_ps, mp_sb[:, kg, :], xt_t[:, kt, :],
                        start=(kg == 0), stop=(kg == NT - 1),
                        skip_group_check=True)
            # remaining finalize consts, then the x stream, all on the same
            # SWDGE queue AFTER the xT stream: pooling (and the finalize it
            # gates) isn't starved by x traffic, while x arrivals still
            # comfortably lead pass-2 consumption.
            wv_sb = consts.tile([128, KT, C], bf)
            nc.gpsimd.dma_start(out=wv_sb, in_=wvv)
            bk_sb = consts.tile([1, O], bf)
            nc.gpsimd.dma_start(out=bk_sb, in_=bk_d)
            bv_sb = consts.tile([1, C], bf)
            nc.gpsimd.dma_start(out=bv_sb, in_=bv_d)
            for ci in range(NBIG):
                nc.gpsimd.dma_start(out=x_sb[:, ci, :, :], in_=xv[:, ci, :, :])

            # ---------------- finalize pools ----------------
            nc.scalar.copy(pooledT_sb, pooledT_ps)
            for c in range(KT):
                tr_ps = p1ps.tile([128, S], mybir.dt.bfloat16, tag="trps")
                nc.tensor.transpose(
                    tr_ps, pooledT_sb[:, c * 128:(c + 1) * 128],
                    id_sb[:110, :110])
                nc.scalar.copy(pooled_sb[:, c, :], tr_ps)
            kp_ps = p1ps.tile([128, MT_O, 128], mybir.dt.float32, tag="kps")
            for m in range(MT_O):
                for k in range(KT):
                    nc.tensor.matmul(
                        kp_ps[:, m, 0:S], wk_sb[:, k, m * 128:(m + 1) * 128],
                        pooled_sb[:, k, :], start=(k == 0), stop=False,
                        skip_group_check=True)
                nc.tensor.matmul(
                    kp_ps[:, m, 0:S], bk_sb[0:1, m * 128:(m + 1) * 128],
                    ones_sb[0:1, 0:S], start=False, stop=True,
                    skip_group_check=True)
            nc.scalar.copy(k_pool_sb, kp_ps[:, :, 0:S])
            # attnT = (Wq.T k_pool).T @ x + (k_pool.T bq) 1^T
            t_ps = p1ps.tile([110, 128], mybir.dt.float32, tag="tps")
            for kt in range(MT_O):
                nc.tensor.matmul(t_ps[:, 0:1], k_pool_sb[:, kt, :],
                                 bq_sb[:, kt, :],
                                 start=(kt == 0), stop=(kt == MT_O - 1),
                                 skip_group_check=True)
            t_sb = consts.tile([110, 1], mybir.dt.float32)
            nc.scalar.copy(t_sb, t_ps[:, 0:1])
            kq_sb = consts.tile([128, KT, S], bf)
            for cb in range(KT):
                kq_ps = p1ps.tile([128, 128], mybir.dt.float32, tag="tps")
                for kt in range(MT_O):
                    nc.tensor.matmul(
                        kq_ps[:, 0:S],
                        wq_sb[:, kt, cb * 128:(cb + 1) * 128],
                        k_pool_sb[:, kt, :],
                        start=(kt == 0), stop=(kt == MT_O - 1),
                        skip_group_check=True)
                nc.scalar.copy(kq_sb[:, cb, :], kq_ps[:, 0:S])
            vp_ps = p1ps.tile([110, C], mybir.dt.float32, tag="vps")
            for k in range(KT):
                nc.tensor.matmul(vp_ps, pooled_sb[:, k, :], wv_sb[:, k, :],
                                 start=(k == 0), stop=False,
                                 skip_group_check=True)
            nc.tensor.matmul(vp_ps, ones_sb[0:1, :110], bv_sb,
                             start=False, stop=True, skip_group_check=True)
            nc.scalar.copy(vT_sb, vp_ps)

        # ------- pass 2: folded attention + output, per 512-col chunk ----
        with tc.tile_pool(name="p2ps", bufs=2, space="PSUM") as p2ps, \
             tc.tile_pool(name="pops", bufs=3, space="PSUM") as pops, \
             tc.tile_pool(name="p2sb", bufs=3) as p2sb, \
             tc.tile_pool(name="outp", bufs=4) as outp:
            for ci in range(NSUB):
                cb, sub = divmod(ci, CHUNK // SUB)
                lsl = slice(sub * SUB, (sub + 1) * SUB)
                at_ps = p2ps.tile([110, SUB], mybir.dt.float32, tag="atdb")
                for k in range(KT):
                    nc.tensor.matmul(at_ps, kq_sb[:, k, :],
                                     x_sb[:, cb, k, lsl],
                                     start=(k == 0), stop=(k == KT - 1))
                exp_sb = p2sb.tile([110, SUB], mybir.dt.bfloat16, tag="exp")
                nc.scalar.activation(exp_sb, at_ps, AF.Exp,
                                     bias=t_sb, scale=1.0)
                db_ps = p2ps.tile([110, SUB], mybir.dt.float32, tag="atdb")
                nc.tensor.matmul(db_ps, ones_sb[:110, :110], exp_sb,
                                 start=True, stop=True)
                recip_sb = p2sb.tile([110, SUB], mybir.dt.float32, tag="recip")
                nc.vector.reciprocal_approx_fast(recip_sb, db_ps)
                attn_sb = p2sb.tile([110, SUB], mybir.dt.bfloat16, tag="attn")
                with nc.allow_low_precision("softmax weights tolerate bf16"):
                    nc.gpsimd.tensor_mul(attn_sb, exp_sb, recip_sb)
                out_t = outp.tile([128, KT, SUB], mybir.dt.float32, tag="out")
                # c 0,1: residual fused into the psum->sbuf move on DVE
                o_psa = pops.tile([128, 2, SUB], mybir.dt.float32, tag="ops")
                for c in range(2):
                    nc.tensor.matmul(o_psa[:, c, :],
                                     vT_sb[:, c * 128:(c + 1) * 128],
                                     attn_sb, start=True, stop=True,
                                     skip_group_check=True)
                with nc.allow_low_precision("fp32 psum + bf16 residual"):
                    nc.vector.tensor_add(out_t[:, 0:2, :], o_psa,
                                         x_sb[:, cb, 0:2, lsl])
                # c 2,3: residual via identity matmul, copy on ACT
                o_psb = pops.tile([128, 2, SUB], mybir.dt.float32, tag="ops")
                for c in range(2, KT):
                    nc.tensor.matmul(o_psb[:, c - 2, :],
                                     vT_sb[:, c * 128:(c + 1) * 128],
                                     attn_sb, start=True, stop=False,
                                     skip_group_check=True)
                    nc.tensor.matmul(o_psb[:, c - 2, :], id_sb,
                                     x_sb[:, cb, c, lsl],
                                     start=False, stop=True,
                                     skip_group_check=True)
                nc.scalar.copy(out_t[:, 2:4, :], o_psb)
                nc.gpsimd.dma_start(out=outv[:, ci, :, :], in_=out_t)

    nc.compile()
    return nc


_NC_CACHE = None


def _get_nc() -> bacc.Bacc:
    global _NC_CACHE
    if _NC_CACHE is None:
        _NC_CACHE = build_nc()
    return _NC_CACHE


def _prep_in_maps(x, Wq, bq, Wk, bk, Wv, bv):
    shared = {
        "mp_st": _stage(_build_pool_matrix().astype(BF16)),
        "wq_st": _stage(np.ascontiguousarray(Wq).astype(BF16)),
        "wk_st": _stage(np.ascontiguousarray(Wk.T).astype(BF16)),
        "wv_st": _stage(np.ascontiguousarray(Wv.T).astype(BF16)),
        "bq_col": np.ascontiguousarray(bq.reshape(O, 1)).astype(BF16),
        "bk_bf": np.ascontiguousarray(bk.reshape(1, O)).astype(BF16),
        "bv_bf": np.ascontiguousarray(bv.reshape(1, C)).astype(BF16),
        "ones_bf": np.ones((128, 512), dtype=BF16),
        "ident_bf": np.eye(128, dtype=np.float32).astype(BF16),
    }
    in_maps = []
    for i in range(NCORES):
        xi = np.ascontiguousarray(x[i].reshape(C, N))
        xi_bf = xi.astype(BF16)
        # x: [p, ci, g, nn] with g the 128-channel block
        x_st = np.ascontiguousarray(
            xi_bf.reshape(KT, 128, NBIG, CHUNK).transpose(1, 2, 0, 3)
            .reshape(128, -1))
        # xT: [p, ci, kt, c]
        xT_st = np.ascontiguousarray(
            xi_bf.T.reshape(NBIG, 8, 128, C).transpose(2, 0, 1, 3)
            .reshape(128, -1))
        m = dict(shared)
        m["x_st"] = x_st
        m["xT_st"] = xT_st
        in_maps.append(m)
    return in_maps


def _unstage_out(o: np.ndarray) -> np.ndarray:
    # [128, NSUB*KT*SUB] -> [C, H, W]
    return np.ascontiguousarray(
        o.reshape(128, NSUB, KT, SUB).transpose(2, 0, 1, 3)
        .reshape(C, N)).reshape(C, H, W)


def _install_ntff_hook():
    """The agent image ships no antenv.axon_hooks module, so trace=True
    under axon crashes on import. Recreate the tiny hook-holder module and
    register trn_boot's ctypes NTFF hook so neuron-profile timing works."""
    import sys
    import types
    if "antenv.axon_hooks" in sys.modules:
        return
    mod = types.ModuleType("antenv.axon_hooks")
    holder = {"h": None}
    mod.set_axon_ntff_profile_hook = lambda h: holder.__setitem__("h", h)
    mod.get_axon_ntff_profile_hook = lambda: holder["h"]
    sys.modules["antenv.axon_hooks"] = mod
    try:
        import antenv
        antenv.axon_hooks = mod
    except ImportError:
        pass
    try:
        from trn_agent_boot.trn_boot import _ntff_profile_via_ctypes
        mod.set_axon_ntff_profile_hook(
            _ntff_profile_via_ctypes("/opt/axon/libaxon_pjrt.so"))
    except Exception as e:  # degrade to no profiling
        print(f"ntff hook install failed: {e}")


def _run(trace: bool, **inputs):
    if trace:
        _install_ntff_hook()
        import concourse.bass_utils as bu
        bu.upload_artifacts = lambda tmpdir: tmpdir  # no cloud bucket here
    nc = _get_nc()
    in_maps = _prep_in_maps(
        inputs["x"], inputs["Wq"], inputs["bq"], inputs["Wk"], inputs["bk"],
        inputs["Wv"], inputs["bv"])
    res = run_bass_kernel_spmd(nc, in_maps, core_ids=list(range(NCORES)),
                               trace=trace)
    out = np.stack([
        _unstage_out(np.asarray(res.results[i]["out_st"]))
        for i in range(NCORES)
    ]).astype(np.float32)
    return out, res


def kernel(**inputs) -> np.ndarray:
    out, _ = _run(False, **inputs)
    return out


def kernel_profiled(**inputs):
    out, res = _run(True, **inputs)
    return out, res
